# revision 1
# baseline (speedup 1.0000x reference)
"""MAGNO encoder (GNO radius-graph message passing) on 8 Trainium2 NeuronCores.

Strategy: shard the B*NL = 1024 (batch, latent-query) pairs as 128 per core
(64 latents x 2 batches) -- one partition row per query in the mask/weight
pipeline, and (4 queries x 32 channels) partition packing for the kernel-MLP.

Key algebraic restructuring:
  - MLP layer 1 splits: h1 = gelu(A[j,:] + cq[q,:]) with A = y @ kW1[:2]
    (per-node, precomputed once) and cq = x_q @ kW1[2:] + kb1 (per-query,
    applied via the ACT engine's per-partition bias operand) -- no per-pair
    layer-1 matmul.
  - Layers 2/3 run as single 128x128 matmuls against 4x block-diagonal
    weights, processing 4 queries x 512 nodes x 32 channels per instruction.
  - Radius masks/weights w[q,j] = mask1/cnt1 + mask2/cnt2 are computed once
    per core in [query-partition, node-free] layout with the same fp32 op
    order as the reference (bit-exact d2 -> no boundary flips), then
    broadcast per-tile into the (query,channel) layout with a K=4 selector
    matmul on the otherwise-idle PE.
  - The weighted reduction over nodes fuses into one custom DVE op
    (affine_mul_reduce): acc[p] = sum_j (k + kb3) * (f*w).
"""
import sys

if "/opt/trn_rl_repo" not in sys.path:
    sys.path.insert(0, "/opt/trn_rl_repo")

import numpy as np

B, N, NL, CD, IN_C, C, H = 2, 2048, 512, 2, 16, 32, 32
NCORES = 8
QL = NL // NCORES  # 64 latents per core
NT = QL // 4       # 16 quads per batch
NCHUNK = 4         # 512-node chunks
JC = N // NCHUNK   # 512
RADIUS = 0.07
SCALES = (1.0, 2.0)

_CACHE = {}


# --------------------------------------------------------------------------
# Workaround: this walrus build allows only ONE sync-wait per CTRL
# instruction; TileContext's tail drain carries one wait per outstanding
# semaphore.  Redistribute them across a chain of SP nops.
def _apply_tile_patch(tile_mod, mybir):
    from concourse.vector_clock import ScopedClock

    if getattr(tile_mod.TileContext, "_ant_drain_patched", False):
        return

    def _patched(self, tick_clock, wait_clock):
        probe = self.nc.sync.nop(nofuse=True)
        wait_clock.add_sem_waits(
            probe.ins, ScopedClock({None: tick_clock.global_clock})
        )
        si = probe.ins.sync_info
        waits = list(si.on_wait) if si is not None else []
        if len(waits) > 1:
            probe.ins.sync_info = mybir.SyncInfo(
                on_wait=waits[:1],
                on_update=list(si.on_update) if si.on_update else [],
            )
            for i in range(1, len(waits)):
                n = self.nc.sync.nop(nofuse=True)
                n.ins.sync_info = mybir.SyncInfo(on_wait=[waits[i]], on_update=[])
        self.nc.sync.drain()
        self.nc.all_engine_barrier()
        assert self.sems is not None
        popped = self.nc._tile_sem_poison_stack.pop()
        assert popped is self._sem_poison
        self.nc.clear_and_free_semaphores(list(self.sems.allocated().values()))
        self.nc.all_engine_barrier()

    tile_mod.TileContext._drain_and_barrier = _patched
    tile_mod.TileContext._ant_drain_patched = True


def _split_multi_waits(nc, mybir):
    """Walrus here encodes at most ONE sync-wait per instruction.  Hoist
    extra waits onto same-engine nops inserted just before (engines block
    on queued instructions in order, so semantics are unchanged)."""
    k = 0
    for fn in nc.m.functions:
        for blk in fn.blocks:
            newl = []
            for ins in blk.instructions:
                si = ins.sync_info
                waits = list(si.on_wait) if si is not None else []
                if len(waits) > 1:
                    for w in waits[:-1]:
                        nop = mybir.InstDrain(
                            name=f"antw-{k}", ins=[], outs=[], engine=ins.engine,
                            is_reset_sema=False,
                        )
                        k += 1
                        nop.sync_info = mybir.SyncInfo(on_wait=[w], on_update=[])
                        newl.append(nop)
                    ins.sync_info = mybir.SyncInfo(
                        on_wait=[waits[-1]],
                        on_update=list(si.on_update) if si.on_update else [],
                    )
                newl.append(ins)
            blk.instructions = newl


def build_nc():
    """Build the (input-independent) Bass module for one core."""
    import concourse.bass as bass
    import concourse.tile as tile
    from concourse import mybir

    _apply_tile_patch(tile, mybir)
    f32 = mybir.dt.float32
    AF = mybir.ActivationFunctionType
    OP = mybir.AluOpType

    nc = bass.Bass()
    dp = nc.declare_dram_parameter
    xcT_e = dp("xcT", [B, CD, N], f32, isOutput=False)      # coords, transposed
    pndT_e = dp("pndT", [B, IN_C, N], f32, isOutput=False)  # features, transposed
    latq_e = dp("latq", [128, CD], f32, isOutput=False)     # per-row query coords
    latT_e = dp("latT", [CD, QL], f32, isOutput=False)
    Wlift_e = dp("Wlift", [IN_C, C], f32, isOutput=False)
    kW1x_e = dp("kW1x", [CD, H], f32, isOutput=False)
    kW1q_e = dp("kW1q", [CD, H], f32, isOutput=False)
    W2bd_e = dp("W2bd", [128, 128], f32, isOutput=False)
    W3bd_e = dp("W3bd", [128, 128], f32, isOutput=False)
    Bsel_e = dp("Bsel", [4, 128], f32, isOutput=False)
    rep4_e = dp("rep4", [H, 128], f32, isOutput=False)
    SEL_e = dp("SEL", [4, H, 128], f32, isOutput=False)
    xc4_e = dp("xc4", [2 * B, N], f32, isOutput=False)
    selc_e = dp("selc", [CD, 2 * B, 128], f32, isOutput=False)
    kb1r_e = dp("kb1rep", [128, 1], f32, isOutput=False)
    kb2r_e = dp("kb2rep", [128, 1], f32, isOutput=False)
    kb3r_e = dp("kb3rep", [128, 1], f32, isOutput=False)
    blc_e = dp("bliftc", [C, 1], f32, isOutput=False)
    out_e = dp("out", [128, C], f32, isOutput=True)

    thr = [float(np.float32((RADIUS * s) ** 2)) for s in SCALES]

    with tile.TileContext(nc) as tc:
        with (
            tc.tile_pool(name="const", bufs=1) as cp,
            tc.tile_pool(name="big", bufs=1) as bp,
            tc.tile_pool(name="work", bufs=3) as wp,
            tc.tile_pool(name="w2q", bufs=3) as wqp,
            tc.tile_pool(name="mmp", bufs=2, space="PSUM") as mp,
        ):
            # ---- loads -------------------------------------------------
            def load(pool, shape, src, tag):
                t = pool.tile(shape, f32, tag=tag, name=tag)
                nc.sync.dma_start(t[:], src)
                return t

            latq = load(cp, [128, CD], latq_e[:], "latq")
            latT = load(cp, [CD, QL], latT_e[:], "latT")
            Wlift = load(cp, [IN_C, C], Wlift_e[:], "wlift")
            kW1x = load(cp, [CD, H], kW1x_e[:], "kw1x")
            kW1q = load(cp, [CD, H], kW1q_e[:], "kw1q")
            W2bd = load(cp, [128, 128], W2bd_e[:], "w2bd")
            W3bd = load(cp, [128, 128], W3bd_e[:], "w3bd")
            Bsel = load(cp, [4, 128], Bsel_e[:], "bsel")
            rep4 = load(cp, [H, 128], rep4_e[:], "rep4")
            SEL = [load(cp, [H, 128], SEL_e[g], f"sel{g}") for g in range(4)]
            selc = [load(cp, [2 * B, 128], selc_e[d], f"selc{d}") for d in range(CD)]
            kb1r = load(cp, [128, 1], kb1r_e[:], "kb1r")
            kb2r = load(cp, [128, 1], kb2r_e[:], "kb2r")
            kb3r = load(cp, [128, 1], kb3r_e[:], "kb3r")
            blc = load(cp, [C, 1], blc_e[:], "blc")

            AT4 = [bp.tile([128, N], f32, tag=f"at4_{b}", name=f"at4_{b}") for b in range(B)]
            fT4 = [bp.tile([128, N], f32, tag=f"ft4_{b}", name=f"ft4_{b}") for b in range(B)]
            biasbuf = bp.tile([128, NT], f32, tag="biasbuf", name="biasbuf")
            acccols = bp.tile([128, 128], f32, tag="acccols", name="acccols")
            w_all = bp.tile([128, N], f32, tag="w_all", name="w_all")
            out_sb = bp.tile([128, C], f32, tag="out_sb", name="out_sb")

            with (
                tc.tile_pool(name="pre", bufs=1) as tp,
                tc.tile_pool(name="prep", bufs=2, space="PSUM") as pp,
            ):
                # ---- cq / biasbuf -------------------------------------
                ps = pp.tile([H, QL], f32, tag="pre", name="pre")
                nc.tensor.matmul(ps[:], kW1q[:], latT[:], start=True, stop=True)
                cq_sb = tp.tile([H, QL], f32, tag="cq", name="cq")
                nc.vector.tensor_copy(cq_sb[:], ps[:])
                psb = pp.tile([128, NT], f32, tag="pre", name="pre")
                for g in range(4):
                    nc.tensor.matmul(
                        psb[:], SEL[g][:], cq_sb[:, g::4],
                        start=(g == 0), stop=(g == 3),
                    )
                nc.vector.tensor_scalar_add(biasbuf[:], psb[:], kb1r[:])

                # ---- A = y @ kW1[:2], f = pnd @ Wlift, 4x-replicated --
                for b in range(B):
                    xcT = tp.tile([CD, N], f32, tag="xct", name="xct")
                    nc.sync.dma_start(xcT[:], xcT_e[b])
                    pnd = tp.tile([IN_C, N], f32, tag="pnd", name="pnd")
                    nc.sync.dma_start(pnd[:], pndT_e[b])
                    t32 = tp.tile([H, N], f32, tag="t32", name="t32")
                    for ch in range(NCHUNK):
                        pa = pp.tile([H, JC], f32, tag="pre", name="pre")
                        nc.tensor.matmul(
                            pa[:], kW1x[:], xcT[:, JC * ch:JC * (ch + 1)],
                            start=True, stop=True,
                        )
                        nc.vector.tensor_copy(t32[:, JC * ch:JC * (ch + 1)], pa[:])
                    for ch in range(NCHUNK):
                        pr = pp.tile([128, JC], f32, tag="pre", name="pre")
                        nc.tensor.matmul(
                            pr[:], rep4[:], t32[:, JC * ch:JC * (ch + 1)],
                            start=True, stop=True,
                        )
                        nc.vector.tensor_copy(AT4[b][:, JC * ch:JC * (ch + 1)], pr[:])
                    ft = tp.tile([C, N], f32, tag="t32", name="ft")
                    for ch in range(NCHUNK):
                        pf = pp.tile([C, JC], f32, tag="pre", name="pre")
                        nc.tensor.matmul(
                            pf[:], Wlift[:], pnd[:, JC * ch:JC * (ch + 1)],
                            start=True, stop=True,
                        )
                        nc.vector.tensor_scalar_add(
                            ft[:, JC * ch:JC * (ch + 1)], pf[:], blc[:]
                        )
                    for ch in range(NCHUNK):
                        pr = pp.tile([128, JC], f32, tag="pre", name="pre")
                        nc.tensor.matmul(
                            pr[:], rep4[:], ft[:, JC * ch:JC * (ch + 1)],
                            start=True, stop=True,
                        )
                        nc.vector.tensor_copy(fT4[b][:, JC * ch:JC * (ch + 1)], pr[:])

                # ---- y broadcast + d2 + masks + weights ---------------
                xc4 = tp.tile([2 * B, N], f32, tag="xc4", name="xc4")
                nc.sync.dma_start(xc4[:], xc4_e[:])
                yb = [tp.tile([128, N], f32, tag=f"yb{d}", name=f"yb{d}") for d in range(CD)]
                for d in range(CD):
                    for ch in range(NCHUNK):
                        py = pp.tile([128, JC], f32, tag="pre", name="pre")
                        nc.tensor.matmul(
                            py[:], selc[d][:], xc4[:, JC * ch:JC * (ch + 1)],
                            start=True, stop=True,
                        )
                        nc.vector.tensor_copy(yb[d][:, JC * ch:JC * (ch + 1)], py[:])

                for d in range(CD):
                    nc.vector.tensor_scalar_sub(yb[d][:], yb[d][:], latq[:, d:d + 1])
                    nc.vector.tensor_tensor(yb[d][:], yb[d][:], yb[d][:], OP.mult)
                nc.vector.tensor_tensor(yb[0][:], yb[0][:], yb[1][:], OP.add)
                d2 = yb[0]

                msk = [tp.tile([128, N], f32, tag=f"msk{s}", name=f"msk{s}")
                       for s in range(2)]
                for s in range(2):
                    nc.vector.tensor_scalar(msk[s][:], d2[:], thr[s], None, OP.is_le)
                    cnt = tp.tile([128, 1], f32, tag=f"cnt{s}", name=f"cnt{s}")
                    nc.vector.tensor_reduce(
                        cnt[:], msk[s][:], mybir.AxisListType.X, OP.add
                    )
                    nc.vector.tensor_scalar_max(cnt[:], cnt[:], 1.0)
                    rc = tp.tile([128, 1], f32, tag=f"rc{s}", name=f"rc{s}")
                    nc.vector.reciprocal(rc[:], cnt[:])
                    nc.vector.tensor_scalar_mul(msk[s][:], msk[s][:], rc[:])
                nc.vector.tensor_tensor(w_all[:], msk[0][:], msk[1][:], OP.add)

            # ---- main loop: 32 quads x 4 chunks ------------------------
            for qd in range(2 * NT):
                b, t = qd // NT, qd % NT
                w2q = wqp.tile([4, N], f32, tag="w2q", name="w2q")
                nc.sync.dma_start(w2q[:], w_all[QL * b + 4 * t: QL * b + 4 * t + 4, :])
                for ch in range(NCHUNK):
                    sl = slice(JC * ch, JC * (ch + 1))
                    h1 = wp.tile([128, JC], f32, tag="h1", name="h1")
                    nc.scalar.activation(
                        h1[:], AT4[b][:, sl], AF.Gelu_apprx_tanh,
                        bias=biasbuf[:, t:t + 1], scale=1.0,
                    )
                    p2 = mp.tile([128, JC], f32, tag="p2", name="p2")
                    nc.tensor.matmul(p2[:], W2bd[:], h1[:], start=True, stop=True)
                    h2 = wp.tile([128, JC], f32, tag="h2", name="h2")
                    nc.scalar.activation(
                        h2[:], p2[:], AF.Gelu_apprx_tanh, bias=kb2r[:], scale=1.0
                    )
                    p3 = mp.tile([128, JC], f32, tag="p3", name="p3")
                    nc.tensor.matmul(p3[:], W3bd[:], h2[:], start=True, stop=True)
                    pw = mp.tile([128, JC], f32, tag="pw", name="pw")
                    nc.tensor.matmul(pw[:], Bsel[:], w2q[0:4, sl], start=True, stop=True)
                    fw = wp.tile([128, JC], f32, tag="fw", name="fw")
                    nc.vector.tensor_tensor(fw[:], fT4[b][:, sl], pw[:], OP.mult)
                    scr = wp.tile([128, JC], f32, tag="scr", name="scr")
                    col = 4 * qd + ch
                    nc.vector.scalar_tensor_tensor(
                        scr[:], p3[:], kb3r[:], fw[:],
                        OP.add, OP.mult, accum_out=acccols[:, col:col + 1],
                    )

            # ---- finalize ---------------------------------------------
            nc.vector.tensor_reduce(
                out_sb[:],
                acccols[:].rearrange("p (a c) -> p a c", a=C),
                mybir.AxisListType.X, OP.add,
            )
            nc.sync.dma_start(out_e[:], out_sb[:])
    _split_multi_waits(nc, mybir)
    return nc


def _host_inputs(x_coord, pndata, latent_tokens_coord,
                 W_lift, b_lift, kW1, kb1, kW2, kb2, kW3, kb3):
    """Common (core-independent) input arrays + per-core latent slices."""
    f = np.float32
    a = lambda x: np.ascontiguousarray(np.asarray(x, dtype=f))

    def bd4(w):
        o = np.zeros((128, 128), f)
        for g in range(4):
            o[32 * g:32 * g + 32, 32 * g:32 * g + 32] = w
        return o

    Bsel = np.zeros((4, 128), f)
    for g in range(4):
        Bsel[g, 32 * g:32 * g + 32] = 1.0
    rep4 = np.zeros((H, 128), f)
    SEL = np.zeros((4, H, 128), f)
    for g in range(4):
        for c in range(H):
            rep4[c, 32 * g + c] = 1.0
            SEL[g, c, 32 * g + c] = 1.0
    xc = np.asarray(x_coord, dtype=f)
    xc4 = np.zeros((2 * B, N), f)
    selc = np.zeros((CD, 2 * B, 128), f)
    for b_ in range(B):
        for d_ in range(CD):
            xc4[2 * b_ + d_] = xc[b_, :, d_]
            selc[d_, 2 * b_ + d_, QL * b_: QL * (b_ + 1)] = 1.0

    common = {
        "xcT": a(np.transpose(np.asarray(x_coord), (0, 2, 1))),
        "pndT": a(np.transpose(np.asarray(pndata), (0, 2, 1))),
        "Wlift": a(W_lift),
        "kW1x": a(np.asarray(kW1)[:CD]),
        "kW1q": a(np.asarray(kW1)[CD:]),
        "W2bd": bd4(a(kW2)),
        "W3bd": bd4(a(kW3)),
        "Bsel": Bsel, "rep4": rep4, "SEL": SEL, "xc4": xc4, "selc": selc,
        "kb1rep": np.tile(a(kb1), 4)[:, None].copy(),
        "kb2rep": np.tile(a(kb2), 4)[:, None].copy(),
        "kb3rep": np.tile(a(kb3), 4)[:, None].copy(),
        "bliftc": a(b_lift)[:, None].copy(),
    }
    lat = a(latent_tokens_coord)
    in_maps = []
    for k in range(NCORES):
        sl = lat[QL * k: QL * (k + 1)]
        m = dict(common)
        m["latq"] = np.ascontiguousarray(np.tile(sl, (B, 1)))
        m["latT"] = np.ascontiguousarray(sl.T)
        in_maps.append(m)
    return in_maps


def _assemble(results):
    out = np.zeros((B, NL, C), np.float32)
    for k in range(NCORES):
        oc = results[k]["out"]                     # [128, 32]: row 32*qg+c, col 16*b+t
        v = oc.reshape(4, C, B, NT)                # (qg, c, b, t)
        v = v.transpose(2, 3, 0, 1).reshape(B, QL, C)  # q_local = 4*t + qg
        out[:, QL * k: QL * (k + 1), :] = v
    return out


def kernel(**inputs):
    from concourse.bass_utils import run_bass_kernel_spmd

    if "nc" not in _CACHE:
        _CACHE["nc"] = build_nc()
    nc = _CACHE["nc"]
    in_maps = _host_inputs(**inputs)
    res = run_bass_kernel_spmd(nc, in_maps, list(range(NCORES)), trace=False)
    return _assemble(res.results)

